# revision 40
# baseline (speedup 1.0000x reference)
"""Trainium2 Bass kernel for nn_MultiHeadAttention (B=4, S=2048, D=512, H=8).

Sharding: 2D tensor x data parallel - core c = (hp=c//2, bp=c%2) owns heads
{2hp, 2hp+1} and batches {2bp, 2bp+1}. Each core computes q/k/v projections
for its two heads over its two batches (x^T shipped bf16, 4MB/core), runs
attention per (batch, head) with both heads packed on SBUF partition halves,
and ships per-head unnormalized partial out-projections plus softmax
denominators (riding row 64 of each O^T tile via the ones-column of V_aug);
the host divides by denominators, sums the 16 (core, head) partials, and adds
the commuting biases (bo, bv@Wo). All on-core compute is bf16.

Engine plan (emission order IS the per-engine execution order):
  - PE: q then k projected per 128-dim head-pair slice (full M=128), V
    projected with M=128 (both heads in one pass), PE-transposed into the
    [key, dh] AV layout, row-quadrant S^T (tile_position (h*64, 0)) so the
    two heads' score matmuls overlap, AV with per-head ones columns of V_aug
    producing softmax denominators in PSUM row 64, per-head out-projection.
  - ACT: exclusively exp(S/8) on [128,1024] tiles - the ~144us bottleneck;
    everything else is scheduled to hide under it.
  - DVE: all PSUM evacuations.
Attention is one flat software-pipelined stream over 128 (b,qq,kt) steps:
per step the PE does [S^T(i+1), filler, AV(i-1)] so AV never waits on its
exp. xT arrives as 4 small blk0 DMAs (to start compute ~2us in) plus big
[128,1536]/[128,2048] contiguous transfers for the rest.
"""
import numpy as np

import concourse.bass as bass
import concourse.mybir as mybir
import concourse.tile as tile
from concourse import bacc
from concourse.bass_utils import run_bass_kernel_spmd

B, S, D = 4, 2048, 512
H, DH = 8, 64
NCORES = 8
F32 = mybir.dt.float32
BF16 = mybir.dt.bfloat16
AF = mybir.ActivationFunctionType

NB = 2                  # local batches per core
NKT = S // 128          # 16 key tiles per batch
NQB = S // 512          # 4 query blocks per batch
NCH = D // 128          # 4 dm chunks

_NC_CACHE = {}


def build_kernel():
    nc = bacc.Bacc("TRN2", target_bir_lowering=False, debug=False)

    xT = nc.dram_tensor("xT", [NB, D, S], BF16, kind="ExternalInput")
    # per-chunk [wq(128) | wk(128)] (4*256) | wv per-chunk (4*128) | ident
    wpack = nc.dram_tensor("wpack", [128, 1664], BF16, kind="ExternalInput")
    wo = nc.dram_tensor("wo", [DH, 2 * D], BF16, kind="ExternalInput")
    bqk = nc.dram_tensor("bqk", [128, 2], F32, kind="ExternalInput")
    out = nc.dram_tensor("out", [2, NB * S, D], BF16, kind="ExternalOutput")
    dnm = nc.dram_tensor("dnm", [2, NB, S], BF16, kind="ExternalOutput")

    with tile.TileContext(nc) as tc:
        with (
            tc.tile_pool(name="consts", bufs=1) as consts,
            tc.tile_pool(name="xtp", bufs=2) as xtp,
            tc.tile_pool(name="qkp", bufs=4) as qkp,
            tc.tile_pool(name="vtp", bufs=2) as vtp,
            tc.tile_pool(name="vp", bufs=2) as vp,
            tc.tile_pool(name="ptp", bufs=4) as ptp,
            tc.tile_pool(name="otp", bufs=6) as otp,
            tc.tile_pool(name="sop", bufs=4) as sopp,
            tc.tile_pool(name="psA", bufs=2, space="PSUM") as psA,   # pst [128,1024] f32
            tc.tile_pool(name="psO", bufs=2, space="PSUM") as psO,   # po [65,512] f32
            tc.tile_pool(name="psM", bufs=2, space="PSUM") as psM,   # misc [128,512] f32
        ):
            bqk_sb = consts.tile([128, 2], F32)
            wp_sb = consts.tile([128, 1664], BF16)
            wo_sb = consts.tile([DH, 2 * D], BF16)
            warm = consts.tile([128, 1], BF16)
            scratch = consts.tile([128, 512], BF16)
            nc.sync.dma_start(out=bqk_sb[:], in_=bqk[:])
            nc.scalar.dma_start(out=wp_sb[:], in_=wpack[:])
            # warmup: pulls the Exp table load (~2.7us) into the kernel head
            nc.scalar.activation(warm[:], bqk_sb[:, 0:1], AF.Exp, scale=0.125)
            # HAM warmup: ~3.4us of dummy matmuls on a memset scratch tile
            # during the weight-DMA wait so the real prep runs at 2.4 GHz
            nc.gpsimd.memset(scratch[:], 0.0)
            pd = psM.tile([128, 512], F32, tag="psM", name="pd_warm")
            for _ in range(8):
                nc.tensor.matmul(pd[:], scratch[:, 0:128], scratch[:],
                                 start=True, stop=True)
            ident = wp_sb[:, 1536:1664]

            def w_q(ci):
                return wp_sb[:, bass.ds(ci * 128, 128)]

            def w_k(ci):
                return wp_sb[:, bass.ds(512 + ci * 128, 128)]

            def w_v(ci):
                return wp_sb[:, bass.ds(1024 + ci * 128, 128)]

            state = {}

            def alloc_b(b):
                st = {"ot": {}}
                st["qt"] = qkp.tile([128, S], BF16, tag="qt", name=f"qt_{b}")
                st["kt"] = qkp.tile([128, S], BF16, tag="kt", name=f"kt_{b}")
                # [V_h0(0:64) | ones(64) | V_h1(65:129) | ones(129)] pad->132
                st["v"] = vp.tile([128, NKT, 132], BF16, tag="v", name=f"v_{b}")
                state[b] = st

            def emit_ones(b):
                # engine-side memset: no DMA-ring descriptors for the
                # strided ones columns of V_aug
                nc.gpsimd.memset(state[b]["v"][:, :, 64:65], 1.0)
                nc.gpsimd.memset(state[b]["v"][:, :, 129:130], 1.0)

            def emit_xt_slice(b, eng, lo, hi):
                # one 3D-AP DMA per column slice (all 4 chunks) in deadline
                # order, all on one FIFO ring so later transfers cannot
                # steal bandwidth from earlier ones
                st = state[b]
                if "xt" not in st:
                    st["xt"] = xtp.tile([128, NCH, S], BF16, tag="xt",
                                        name=f"xt_{b}")
                eng.dma_start(
                    out=st["xt"][:, :, lo:hi],
                    in_=xT[b].rearrange("(c p) s -> p c s", p=128)[:, :, lo:hi],
                )

            gst = {}

            def emit_proj_g(kind, b, blk, g):
                # 2-chunk granule of a 4-chunk projection accumulation so a
                # filler never exceeds the per-step PE budget (~2 matmuls)
                st = state[b]
                sl = bass.ts(blk, 512)
                w, bias, dst = {
                    "q": (w_q, bqk_sb[:, 0:1], "qt"),
                    "k": (w_k, bqk_sb[:, 1:2], "kt"),
                    "v": (w_v, None, None),
                }[kind]
                if g == 0:
                    gst[(kind, b, blk)] = psM.tile(
                        [128, 512], F32, tag="psM", name=f"p{kind}_{b}_{blk}"
                    )
                ps = gst[(kind, b, blk)]
                for ci in (2 * g, 2 * g + 1):
                    nc.tensor.matmul(
                        ps[:], w(ci), st["xt"][:, ci, sl],
                        start=(ci == 0), stop=(ci == NCH - 1),
                    )
                if g == 1:
                    gst.pop((kind, b, blk))
                    if kind == "v":
                        if blk == 0:
                            st["vt"] = vtp.tile(
                                [128, S], BF16, tag="vt", name=f"vt_{b}"
                            )
                        nc.vector.tensor_copy(st["vt"][:, sl], ps[:])
                    else:
                        nc.vector.tensor_scalar_add(st[dst][:, sl], ps[:], bias)

            def emit_vtr_g(b, blk, g):
                # 2 PE transposes + their evac into the AV layout
                st = state[b]
                pvtr = psM.tile([128, 256], BF16, tag="psM",
                                name=f"pvtr_{b}_{blk}_{g}")
                for jj in range(2):
                    j = 2 * g + jj
                    nc.tensor.transpose(
                        pvtr[:, bass.ts(jj, 128)],
                        st["vt"][:, bass.ds(blk * 512 + j * 128, 128)],
                        ident,
                    )
                # [p, (j a c)] -> v[:, blk*4+2g+j, a*65 + c] for c in 0:64
                nc.vector.tensor_copy(
                    st["v"][:, bass.ds(blk * 4 + 2 * g, 2), 0:130]
                    .rearrange("p k (a c) -> p k a c", c=65)[:, :, :, 0:64],
                    pvtr[:].rearrange("p (k a c) -> p k a c", k=2, a=2),
                )

            def emit_prep_q(b, blk):
                emit_proj_g("q", b, blk, 0)
                emit_proj_g("q", b, blk, 1)

            def emit_prep_k(b, blk):
                emit_proj_g("k", b, blk, 0)
                emit_proj_g("k", b, blk, 1)

            def emit_prep_vmm(b, blk):
                emit_proj_g("v", b, blk, 0)
                emit_proj_g("v", b, blk, 1)

            def emit_prep_vtr(b, blk):
                emit_vtr_g(b, blk, 0)
                emit_vtr_g(b, blk, 1)

            # --- software-pipelined attention over a flat (b, qq, kt) stream
            psts = {}
            ptts = {}
            pos = {}

            def emit_st(b, qq, kt_i, i):
                st = state[b]
                pst = psA.tile([128, 1024], F32, tag="psA", name=f"pst_{b}_{qq}_{kt_i}")
                for h in range(2):
                    nc.tensor.matmul(
                        pst[:, bass.ts(h, 512)],
                        st["kt"][h * DH:(h + 1) * DH, bass.ts(kt_i, 128)],
                        st["qt"][h * DH:(h + 1) * DH, bass.ts(qq, 512)],
                        start=True, stop=True,
                        tile_position=(h * DH, 0),
                    )
                psts[i] = pst

            def emit_exp(i):
                ptt = ptp.tile([128, 1024], BF16, tag="pt", name=f"ptt_{i}")
                nc.scalar.activation(ptt[:], psts.pop(i)[:], AF.Exp, scale=0.125)
                ptts[i] = ptt

            def emit_av(b, qq, kt_i, i):
                st = state[b]
                if kt_i == 0:
                    pos[(b, qq)] = [
                        psO.tile([DH + 1, 512], F32, tag="psO", name=f"po{h}_{b}_{qq}")
                        for h in range(2)
                    ]
                po = pos[(b, qq)]
                ptt = ptts.pop(i)
                for h in range(2):
                    nc.tensor.matmul(
                        po[h][:],
                        st["v"][:, kt_i, bass.ds(h * 65, 65)],
                        ptt[:, bass.ts(h, 512)],
                        start=(kt_i == 0), stop=(kt_i == NKT - 1),
                    )

            def emit_po_evac(b, qq):
                st = state[b]
                if qq == 0:
                    for h in range(2):
                        st["ot"][h] = otp.tile(
                            [DH + 1, S], BF16, tag="ot", name=f"ot_{b}_{h}"
                        )
                po = pos.pop((b, qq))
                for h in range(2):
                    nc.vector.tensor_copy(st["ot"][h][:, bass.ts(qq, 512)], po[h][:])

            def emit_op_tt(b, h, tt, evac_eng=None, dma_eng=None):
                st = state[b]
                ot_h = st["ot"][h]
                pop = psM.tile([128, 512], F32, tag="psM", name=f"pop_{b}_{h}_{tt}")
                nc.tensor.matmul(
                    pop[:], ot_h[0:DH, bass.ts(tt, 128)],
                    wo_sb[:, bass.ds(h * D, D)],
                    start=True, stop=True,
                )
                so = sopp.tile([128, 512], BF16, tag="so", name=f"so_{b}_{h}_{tt}")
                if evac_eng is nc.scalar:
                    nc.scalar.copy(so[:], pop[:])
                else:
                    nc.vector.tensor_copy(so[:], pop[:])
                if dma_eng is None:
                    dma_eng = nc.gpsimd if (b * 32 + h * NKT + tt) % 2 == 0 else nc.sync
                dma_eng.dma_start(
                    out=out[h, bass.ds(b * S + tt * 128, 128), :], in_=so[:]
                )

            def emit_dnm_dma(b, h):
                nc.gpsimd.dma_start(
                    out=dnm[h, b:b + 1, :], in_=state[b]["ot"][h][DH:DH + 1, :]
                )

            # ---------------- emission schedule ----------------
            import functools
            P = functools.partial
            alloc_b(0)
            alloc_b(1)
            # head: all xT on the sync ring in deadline order (per-queue
            # FIFO = priority); weights on scalar contend only with b0-head
            emit_xt_slice(0, nc.sync, 0, 1024)
            emit_ones(0)
            emit_ones(1)
            emit_xt_slice(0, nc.sync, 1024, 1536)
            emit_xt_slice(0, nc.sync, 1536, 2048)
            nc.scalar.dma_start(out=wo_sb[:], in_=wo[:])
            emit_xt_slice(1, nc.sync, 0, 2048)

            # pre-stream head: blk0 of q/k/v for batch 0 + k/vmm/q blk1
            emit_prep_q(0, 0)
            emit_prep_k(0, 0)
            emit_prep_vmm(0, 0)
            emit_prep_vtr(0, 0)
            emit_prep_k(0, 1)
            emit_prep_vmm(0, 1)
            emit_prep_q(0, 1)

            # fillers as 2-matmul granules staged by earliest-allowed step;
            # k/v before q since S^T consumes all key blocks in 16 steps
            fill = []

            def stage(s, fn, *a):
                fill.append((s, P(fn, *a)))

            stage(1, emit_vtr_g, 0, 1, 0)
            stage(2, emit_vtr_g, 0, 1, 1)
            stage(3, emit_proj_g, "k", 0, 2, 0)
            stage(4, emit_proj_g, "k", 0, 2, 1)
            stage(5, emit_proj_g, "v", 0, 2, 0)
            stage(6, emit_proj_g, "v", 0, 2, 1)
            stage(7, emit_vtr_g, 0, 2, 0)
            stage(8, emit_vtr_g, 0, 2, 1)
            stage(9, emit_proj_g, "k", 0, 3, 0)
            stage(10, emit_proj_g, "k", 0, 3, 1)
            stage(11, emit_proj_g, "v", 0, 3, 0)
            stage(12, emit_proj_g, "v", 0, 3, 1)
            stage(12, emit_vtr_g, 0, 3, 0)
            stage(13, emit_vtr_g, 0, 3, 1)
            stage(25, emit_proj_g, "q", 0, 2, 0)
            stage(26, emit_proj_g, "q", 0, 2, 1)
            stage(27, emit_proj_g, "q", 0, 3, 0)
            stage(28, emit_proj_g, "q", 0, 3, 1)
            s = 47
            for kind, blk in [("k", 0), ("q", 0), ("v", 0), ("t", 0),
                              ("k", 1), ("v", 1), ("t", 1),
                              ("k", 2), ("v", 2), ("t", 2),
                              ("k", 3), ("v", 3), ("t", 3),
                              ("q", 1), ("q", 2), ("q", 3)]:
                for g in range(2):
                    if kind == "t":
                        stage(s, emit_vtr_g, 1, blk, g)
                    else:
                        stage(s, emit_proj_g, kind, 1, blk, g)
                    s += 1

            units = [(b, qq, kt) for b in range(NB) for qq in range(NQB)
                     for kt in range(NKT)]
            NSTEP = len(units)
            emit_st(*units[0], 0)
            for i in range(NSTEP):
                emit_exp(i)
                if i + 1 < NSTEP:
                    emit_st(*units[i + 1], i + 1)
                if i >= 1:
                    b, qq, kt = units[i - 1]
                    emit_av(b, qq, kt, i - 1)
                    if kt == NKT - 1:
                        emit_po_evac(b, qq)
                        for h in range(2):
                            for tt in range(qq * 4, qq * 4 + 4):
                                fill.append((0, P(emit_op_tt, b, h, tt)))
                        if qq == NQB - 1:
                            for h in range(2):
                                fill.append((0, P(emit_dnm_dma, b, h)))
                npop = 2 if i == 12 or i >= NSTEP - 12 else 1
                for _ in range(npop):
                    # pop the first STAGE-READY entry in list order: staged
                    # preps (listed first) take precedence at their stage,
                    # stage-0 out-projections backfill the idle slots
                    for idx in range(len(fill)):
                        if fill[idx][0] <= i:
                            fill.pop(idx)[1]()
                            break
                    else:
                        break
            b, qq, kt = units[NSTEP - 1]
            emit_av(b, qq, kt, NSTEP - 1)
            while fill:
                fill.pop(0)[1]()
            # tail: fine-grained evac of the last qq; ACT (done with exps)
            # takes half the final PSUM->SBUF copies off DVE
            st = state[b]
            po = pos.pop((b, qq))
            tail_engs = [nc.gpsimd, nc.sync, nc.scalar]
            for tt_rel in range(4):
                tt = qq * 4 + tt_rel
                dsl = bass.ds(qq * 512 + tt_rel * 128, 128)
                nc.scalar.copy(st["ot"][0][:, dsl], po[0][:, bass.ts(tt_rel, 128)])
                nc.vector.tensor_copy(
                    st["ot"][1][:, dsl], po[1][:, bass.ts(tt_rel, 128)]
                )
                for h in range(2):
                    emit_op_tt(b, h, tt, evac_eng=nc.scalar if h == 0 else None,
                               dma_eng=tail_engs[(tt_rel * 2 + h) % 3])
            for h in range(2):
                emit_dnm_dma(b, h)

    nc.compile()
    return nc


def kernel(x, Wq, bq, Wk, bk, Wv, bv, Wo, bo):
    import ml_dtypes
    BF = ml_dtypes.bfloat16
    x = np.asarray(x, dtype=np.float32)
    xT = np.ascontiguousarray(np.transpose(x, (0, 2, 1))).astype(BF)
    Wq = np.asarray(Wq, dtype=np.float32)
    Wk = np.asarray(Wk, dtype=np.float32)
    Wv = np.asarray(Wv, dtype=np.float32)
    Wo = np.asarray(Wo, dtype=np.float32)
    bq = np.asarray(bq, dtype=np.float32)
    bk = np.asarray(bk, dtype=np.float32)
    bv = np.asarray(bv, dtype=np.float32)
    bo = np.asarray(bo, dtype=np.float32)

    if "nc" not in _NC_CACHE:
        _NC_CACHE["nc"] = build_kernel()
    nc = _NC_CACHE["nc"]

    eye = np.eye(128, dtype=np.float32)

    in_maps = []
    for c in range(NCORES):
        hp, bp = c // 2, c % 2
        hs = slice(hp * 128, (hp + 1) * 128)
        # per-chunk [128 rows, 128 cols] blocks: all wq, then wk, then wv
        wq = np.concatenate(
            [Wq[ci * 128:(ci + 1) * 128, hs] for ci in range(NCH)], axis=1
        )
        wk = np.concatenate(
            [Wk[ci * 128:(ci + 1) * 128, hs] for ci in range(NCH)], axis=1
        )
        wv = np.concatenate(
            [Wv[ci * 128:(ci + 1) * 128, hs] for ci in range(NCH)], axis=1
        )
        wp = np.concatenate([wq, wk, wv, eye], axis=1)
        wo2 = np.concatenate(
            [Wo[hp * 128:hp * 128 + 64, :], Wo[hp * 128 + 64:hp * 128 + 128, :]],
            axis=1,
        )
        in_maps.append({
            "xT": np.ascontiguousarray(xT[2 * bp:2 * bp + 2]),
            "wpack": np.ascontiguousarray(wp).astype(BF),
            "wo": np.ascontiguousarray(wo2).astype(BF),
            "bqk": np.ascontiguousarray(
                np.stack([bq[hs], bk[hs]], axis=1)).astype(np.float32),
        })

    res = run_bass_kernel_spmd(nc, in_maps, list(range(NCORES)))

    acc = np.zeros((B, S, D), dtype=np.float32)
    for c in range(NCORES):
        hp, bp = c // 2, c % 2
        o = np.asarray(res.results[c]["out"]).astype(np.float32)
        o = o.reshape(2, NB, S, D)
        d = np.asarray(res.results[c]["dnm"]).astype(np.float32)
        for h in range(2):
            for lb in range(NB):
                acc[2 * bp + lb] += o[h, lb] / d[h, lb][:, None]
    # biases that commute with the head-reduction, applied at gather time
    acc += bo[None, :] + (bv @ Wo)[None, :]
    return acc


# revision 43
# speedup vs baseline: 1.0076x; 1.0076x over previous
"""Trainium2 Bass kernel for nn_MultiHeadAttention (B=4, S=2048, D=512, H=8).

Sharding: 2D tensor x data parallel - core c = (hp=c//2, bp=c%2) owns heads
{2hp, 2hp+1} and batches {2bp, 2bp+1}. Each core computes q/k/v projections
for its two heads over its two batches (x^T shipped bf16, 4MB/core), runs
attention per (batch, head) with both heads packed on SBUF partition halves,
and ships per-head unnormalized partial out-projections plus softmax
denominators (riding row 64 of each O^T tile via the ones-column of V_aug);
the host divides by denominators, sums the 16 (core, head) partials, and adds
the commuting biases (bo, bv@Wo). All on-core compute is bf16.

Engine plan (emission order IS the per-engine execution order):
  - PE: q then k projected per 128-dim head-pair slice (full M=128), V
    projected with M=128 (both heads in one pass), PE-transposed into the
    [key, dh] AV layout, row-quadrant S^T (tile_position (h*64, 0)) so the
    two heads' score matmuls overlap, AV with per-head ones columns of V_aug
    producing softmax denominators in PSUM row 64, per-head out-projection.
  - ACT: exclusively exp(S/8) on [128,1024] tiles - the ~144us bottleneck;
    everything else is scheduled to hide under it.
  - DVE: all PSUM evacuations.
Attention is one flat software-pipelined stream over 128 (b,qq,kt) steps:
per step the PE does [S^T(i+1), filler, AV(i-1)] so AV never waits on its
exp. xT arrives as 4 small blk0 DMAs (to start compute ~2us in) plus big
[128,1536]/[128,2048] contiguous transfers for the rest.
"""
import numpy as np

import concourse.bass as bass
import concourse.mybir as mybir
import concourse.tile as tile
from concourse import bacc
from concourse.bass_utils import run_bass_kernel_spmd

B, S, D = 4, 2048, 512
H, DH = 8, 64
NCORES = 8
F32 = mybir.dt.float32
BF16 = mybir.dt.bfloat16
AF = mybir.ActivationFunctionType

NB = 2                  # local batches per core
NKT = S // 128          # 16 key tiles per batch
NQB = S // 512          # 4 query blocks per batch
NCH = D // 128          # 4 dm chunks

_NC_CACHE = {}


def build_kernel():
    nc = bacc.Bacc("TRN2", target_bir_lowering=False, debug=False)

    xT = nc.dram_tensor("xT", [NB, D, S], BF16, kind="ExternalInput")
    # per-chunk [wq(128) | wk(128)] (4*256) | wv per-chunk (4*128) | ident
    wpack = nc.dram_tensor("wpack", [128, 1664], BF16, kind="ExternalInput")
    wo = nc.dram_tensor("wo", [DH, 2 * D], BF16, kind="ExternalInput")
    bqk = nc.dram_tensor("bqk", [128, 2], F32, kind="ExternalInput")
    out = nc.dram_tensor("out", [2, NB * S, D], BF16, kind="ExternalOutput")
    dnm = nc.dram_tensor("dnm", [2, NB, S], BF16, kind="ExternalOutput")

    with tile.TileContext(nc) as tc:
        with (
            tc.tile_pool(name="consts", bufs=1) as consts,
            tc.tile_pool(name="xtp", bufs=2) as xtp,
            tc.tile_pool(name="qkp", bufs=4) as qkp,
            tc.tile_pool(name="vtp", bufs=2) as vtp,
            tc.tile_pool(name="vp", bufs=2) as vp,
            tc.tile_pool(name="ptp", bufs=4) as ptp,
            tc.tile_pool(name="otp", bufs=6) as otp,
            tc.tile_pool(name="sop", bufs=4) as sopp,
            tc.tile_pool(name="psA", bufs=2, space="PSUM") as psA,   # pst [128,1024] f32
            tc.tile_pool(name="psO", bufs=2, space="PSUM") as psO,   # po [65,512] f32
            tc.tile_pool(name="psM", bufs=2, space="PSUM") as psM,   # misc [128,512] f32
        ):
            bqk_sb = consts.tile([128, 2], F32)
            wp_sb = consts.tile([128, 1664], BF16)
            wo_sb = consts.tile([DH, 2 * D], BF16)
            warm = consts.tile([128, 1], BF16)
            scratch = consts.tile([128, 512], BF16)
            nc.sync.dma_start(out=bqk_sb[:], in_=bqk[:])
            nc.scalar.dma_start(out=wp_sb[:], in_=wpack[:])
            # warmup: pulls the Exp table load (~2.7us) into the kernel head
            nc.scalar.activation(warm[:], bqk_sb[:, 0:1], AF.Exp, scale=0.125)
            # HAM warmup: ~3.4us of dummy matmuls on a memset scratch tile
            # during the weight-DMA wait so the real prep runs at 2.4 GHz
            nc.gpsimd.memset(scratch[:], 0.0)
            pd = psM.tile([128, 512], F32, tag="psM", name="pd_warm")
            for _ in range(8):
                nc.tensor.matmul(pd[:], scratch[:, 0:128], scratch[:],
                                 start=True, stop=True)
            ident = wp_sb[:, 1536:1664]

            def w_q(ci):
                return wp_sb[:, bass.ds(ci * 128, 128)]

            def w_k(ci):
                return wp_sb[:, bass.ds(512 + ci * 128, 128)]

            def w_v(ci):
                return wp_sb[:, bass.ds(1024 + ci * 128, 128)]

            state = {}

            def alloc_b(b):
                st = {"ot": {}}
                st["qt"] = qkp.tile([128, S], BF16, tag="qt", name=f"qt_{b}")
                st["kt"] = qkp.tile([128, S], BF16, tag="kt", name=f"kt_{b}")
                # [V_h0(0:64) | ones(64) | V_h1(65:129) | ones(129)] pad->132
                st["v"] = vp.tile([128, NKT, 132], BF16, tag="v", name=f"v_{b}")
                state[b] = st

            def emit_ones(b):
                # engine-side memset: no DMA-ring descriptors for the
                # strided ones columns of V_aug
                nc.gpsimd.memset(state[b]["v"][:, :, 64:65], 1.0)
                nc.gpsimd.memset(state[b]["v"][:, :, 129:130], 1.0)

            def emit_xt_slice(b, eng, lo, hi):
                # column slices in deadline order, all on one FIFO ring so
                # later transfers cannot steal bandwidth from earlier ones
                st = state[b]
                if "xt" not in st:
                    st["xt"] = xtp.tile([128, NCH, S], BF16, tag="xt",
                                        name=f"xt_{b}")
                for ci in range(NCH):
                    eng.dma_start(
                        out=st["xt"][:, ci, lo:hi],
                        in_=xT[b, bass.ts(ci, 128), lo:hi],
                    )

            gst = {}

            def emit_proj_g(kind, b, blk, g):
                # 2-chunk granule of a 4-chunk projection accumulation so a
                # filler never exceeds the per-step PE budget (~2 matmuls)
                st = state[b]
                sl = bass.ts(blk, 512)
                w, bias, dst = {
                    "q": (w_q, bqk_sb[:, 0:1], "qt"),
                    "k": (w_k, bqk_sb[:, 1:2], "kt"),
                    "v": (w_v, None, None),
                }[kind]
                if g == 0:
                    gst[(kind, b, blk)] = psM.tile(
                        [128, 512], F32, tag="psM", name=f"p{kind}_{b}_{blk}"
                    )
                ps = gst[(kind, b, blk)]
                for ci in (2 * g, 2 * g + 1):
                    nc.tensor.matmul(
                        ps[:], w(ci), st["xt"][:, ci, sl],
                        start=(ci == 0), stop=(ci == NCH - 1),
                    )
                if g == 1:
                    gst.pop((kind, b, blk))
                    if kind == "v":
                        if blk == 0:
                            st["vt"] = vtp.tile(
                                [128, S], BF16, tag="vt", name=f"vt_{b}"
                            )
                        nc.vector.tensor_copy(st["vt"][:, sl], ps[:])
                    else:
                        nc.vector.tensor_scalar_add(st[dst][:, sl], ps[:], bias)

            def emit_vtr_g(b, blk, g):
                # 2 PE transposes + their evac into the AV layout
                st = state[b]
                pvtr = psM.tile([128, 256], BF16, tag="psM",
                                name=f"pvtr_{b}_{blk}_{g}")
                for jj in range(2):
                    j = 2 * g + jj
                    nc.tensor.transpose(
                        pvtr[:, bass.ts(jj, 128)],
                        st["vt"][:, bass.ds(blk * 512 + j * 128, 128)],
                        ident,
                    )
                # [p, (j a c)] -> v[:, blk*4+2g+j, a*65 + c] for c in 0:64
                nc.vector.tensor_copy(
                    st["v"][:, bass.ds(blk * 4 + 2 * g, 2), 0:130]
                    .rearrange("p k (a c) -> p k a c", c=65)[:, :, :, 0:64],
                    pvtr[:].rearrange("p (k a c) -> p k a c", k=2, a=2),
                )

            def emit_prep_q(b, blk):
                emit_proj_g("q", b, blk, 0)
                emit_proj_g("q", b, blk, 1)

            def emit_prep_k(b, blk):
                emit_proj_g("k", b, blk, 0)
                emit_proj_g("k", b, blk, 1)

            def emit_prep_vmm(b, blk):
                emit_proj_g("v", b, blk, 0)
                emit_proj_g("v", b, blk, 1)

            def emit_prep_vtr(b, blk):
                emit_vtr_g(b, blk, 0)
                emit_vtr_g(b, blk, 1)

            # --- software-pipelined attention over a flat (b, qq, kt) stream
            psts = {}
            ptts = {}
            pos = {}

            def emit_st(b, qq, kt_i, i):
                st = state[b]
                pst = psA.tile([128, 1024], F32, tag="psA", name=f"pst_{b}_{qq}_{kt_i}")
                for h in range(2):
                    nc.tensor.matmul(
                        pst[:, bass.ts(h, 512)],
                        st["kt"][h * DH:(h + 1) * DH, bass.ts(kt_i, 128)],
                        st["qt"][h * DH:(h + 1) * DH, bass.ts(qq, 512)],
                        start=True, stop=True,
                        tile_position=(h * DH, 0),
                    )
                psts[i] = pst

            def emit_exp(i):
                ptt = ptp.tile([128, 1024], BF16, tag="pt", name=f"ptt_{i}")
                nc.scalar.activation(ptt[:], psts.pop(i)[:], AF.Exp, scale=0.125)
                ptts[i] = ptt

            def emit_av(b, qq, kt_i, i):
                st = state[b]
                if kt_i == 0:
                    pos[(b, qq)] = [
                        psO.tile([DH + 1, 512], F32, tag="psO", name=f"po{h}_{b}_{qq}")
                        for h in range(2)
                    ]
                po = pos[(b, qq)]
                ptt = ptts.pop(i)
                for h in range(2):
                    nc.tensor.matmul(
                        po[h][:],
                        st["v"][:, kt_i, bass.ds(h * 65, 65)],
                        ptt[:, bass.ts(h, 512)],
                        start=(kt_i == 0), stop=(kt_i == NKT - 1),
                    )

            def emit_po_evac(b, qq):
                st = state[b]
                if qq == 0:
                    for h in range(2):
                        st["ot"][h] = otp.tile(
                            [DH + 1, S], BF16, tag="ot", name=f"ot_{b}_{h}"
                        )
                po = pos.pop((b, qq))
                for h in range(2):
                    nc.vector.tensor_copy(st["ot"][h][:, bass.ts(qq, 512)], po[h][:])

            def emit_op_tt(b, h, tt, evac_eng=None, dma_eng=None):
                st = state[b]
                ot_h = st["ot"][h]
                pop = psM.tile([128, 512], F32, tag="psM", name=f"pop_{b}_{h}_{tt}")
                nc.tensor.matmul(
                    pop[:], ot_h[0:DH, bass.ts(tt, 128)],
                    wo_sb[:, bass.ds(h * D, D)],
                    start=True, stop=True,
                )
                so = sopp.tile([128, 512], BF16, tag="so", name=f"so_{b}_{h}_{tt}")
                if evac_eng is nc.scalar:
                    nc.scalar.copy(so[:], pop[:])
                else:
                    nc.vector.tensor_copy(so[:], pop[:])
                if dma_eng is None:
                    dma_eng = nc.gpsimd if (b * 32 + h * NKT + tt) % 2 == 0 else nc.sync
                dma_eng.dma_start(
                    out=out[h, bass.ds(b * S + tt * 128, 128), :], in_=so[:]
                )

            def emit_dnm_dma(b, h):
                nc.gpsimd.dma_start(
                    out=dnm[h, b:b + 1, :], in_=state[b]["ot"][h][DH:DH + 1, :]
                )

            # ---------------- emission schedule ----------------
            import functools
            P = functools.partial
            alloc_b(0)
            alloc_b(1)
            # head: all xT on the sync ring in deadline order (per-queue
            # FIFO = priority); weights on scalar contend only with b0-head
            emit_xt_slice(0, nc.sync, 0, 1024)
            emit_ones(0)
            emit_ones(1)
            emit_xt_slice(0, nc.sync, 1024, 1536)
            emit_xt_slice(0, nc.sync, 1536, 2048)
            nc.scalar.dma_start(out=wo_sb[:], in_=wo[:])
            emit_xt_slice(1, nc.sync, 0, 2048)

            # pre-stream head: blk0 of q/k/v for batch 0 + k/vmm/q blk1
            emit_prep_q(0, 0)
            emit_prep_k(0, 0)
            emit_prep_vmm(0, 0)
            emit_prep_vtr(0, 0)
            emit_prep_k(0, 1)
            emit_prep_vmm(0, 1)
            emit_prep_q(0, 1)

            # fillers as 2-matmul granules staged by earliest-allowed step;
            # k/v before q since S^T consumes all key blocks in 16 steps
            fill = []

            def stage(s, fn, *a):
                fill.append((s, P(fn, *a)))

            stage(1, emit_vtr_g, 0, 1, 0)
            stage(2, emit_vtr_g, 0, 1, 1)
            stage(3, emit_proj_g, "k", 0, 2, 0)
            stage(4, emit_proj_g, "k", 0, 2, 1)
            stage(5, emit_proj_g, "v", 0, 2, 0)
            stage(6, emit_proj_g, "v", 0, 2, 1)
            stage(7, emit_vtr_g, 0, 2, 0)
            stage(8, emit_vtr_g, 0, 2, 1)
            stage(9, emit_proj_g, "k", 0, 3, 0)
            stage(10, emit_proj_g, "k", 0, 3, 1)
            stage(11, emit_proj_g, "v", 0, 3, 0)
            stage(12, emit_proj_g, "v", 0, 3, 1)
            stage(12, emit_vtr_g, 0, 3, 0)
            stage(13, emit_vtr_g, 0, 3, 1)
            stage(25, emit_proj_g, "q", 0, 2, 0)
            stage(26, emit_proj_g, "q", 0, 2, 1)
            stage(27, emit_proj_g, "q", 0, 3, 0)
            stage(28, emit_proj_g, "q", 0, 3, 1)
            s = 47
            for kind, blk in [("k", 0), ("q", 0), ("v", 0), ("t", 0),
                              ("k", 1), ("v", 1), ("t", 1),
                              ("k", 2), ("v", 2), ("t", 2),
                              ("k", 3), ("v", 3), ("t", 3),
                              ("q", 1), ("q", 2), ("q", 3)]:
                for g in range(2):
                    if kind == "t":
                        stage(s, emit_vtr_g, 1, blk, g)
                    else:
                        stage(s, emit_proj_g, kind, 1, blk, g)
                    s += 1

            units = [(b, qq, kt) for b in range(NB) for qq in range(NQB)
                     for kt in range(NKT)]
            NSTEP = len(units)
            emit_st(*units[0], 0)
            for i in range(NSTEP):
                emit_exp(i)
                if i + 1 < NSTEP:
                    emit_st(*units[i + 1], i + 1)
                if i >= 1:
                    b, qq, kt = units[i - 1]
                    emit_av(b, qq, kt, i - 1)
                    if kt == NKT - 1:
                        emit_po_evac(b, qq)
                        for h in range(2):
                            for tt in range(qq * 4, qq * 4 + 4):
                                fill.append((0, P(emit_op_tt, b, h, tt)))
                        if qq == NQB - 1:
                            for h in range(2):
                                fill.append((0, P(emit_dnm_dma, b, h)))
                if i == NSTEP - 6:
                    # keep the PE warm through the stream end so the tail's
                    # out-projections run at full clock
                    def emit_dummy(k):
                        pdw = psM.tile([128, 512], F32, tag="psM",
                                       name=f"pdw_{k}")
                        for _ in range(2):
                            nc.tensor.matmul(pdw[:], scratch[:, 0:128],
                                             scratch[:], start=True, stop=True)
                    for s in range(NSTEP - 5, NSTEP - 1):
                        fill.append((s, P(emit_dummy, s)))
                npop = 2 if i == 12 or i >= NSTEP - 12 else 1
                for _ in range(npop):
                    # pop the first STAGE-READY entry in list order: staged
                    # preps (listed first) take precedence at their stage,
                    # stage-0 out-projections backfill the idle slots
                    for idx in range(len(fill)):
                        if fill[idx][0] <= i:
                            fill.pop(idx)[1]()
                            break
                    else:
                        break
            b, qq, kt = units[NSTEP - 1]
            emit_av(b, qq, kt, NSTEP - 1)
            while fill:
                fill.pop(0)[1]()
            # tail: fine-grained evac of the last qq; ACT (done with exps)
            # takes half the final PSUM->SBUF copies off DVE
            st = state[b]
            po = pos.pop((b, qq))
            tail_engs = [nc.gpsimd, nc.sync, nc.scalar]
            qsl = bass.ts(qq, 512)
            nc.scalar.copy(st["ot"][0][:, qsl], po[0][:])
            nc.vector.tensor_copy(st["ot"][1][:, qsl], po[1][:])
            for tt_rel in range(4):
                tt = qq * 4 + tt_rel
                for h in range(2):
                    emit_op_tt(b, h, tt, evac_eng=nc.scalar if h == 0 else None,
                               dma_eng=tail_engs[(tt_rel * 2 + h) % 3])
            for h in range(2):
                emit_dnm_dma(b, h)

    nc.compile()
    return nc


def kernel(x, Wq, bq, Wk, bk, Wv, bv, Wo, bo):
    import ml_dtypes
    BF = ml_dtypes.bfloat16
    x = np.asarray(x, dtype=np.float32)
    xT = np.ascontiguousarray(np.transpose(x, (0, 2, 1))).astype(BF)
    Wq = np.asarray(Wq, dtype=np.float32)
    Wk = np.asarray(Wk, dtype=np.float32)
    Wv = np.asarray(Wv, dtype=np.float32)
    Wo = np.asarray(Wo, dtype=np.float32)
    bq = np.asarray(bq, dtype=np.float32)
    bk = np.asarray(bk, dtype=np.float32)
    bv = np.asarray(bv, dtype=np.float32)
    bo = np.asarray(bo, dtype=np.float32)

    if "nc" not in _NC_CACHE:
        _NC_CACHE["nc"] = build_kernel()
    nc = _NC_CACHE["nc"]

    eye = np.eye(128, dtype=np.float32)

    in_maps = []
    for c in range(NCORES):
        hp, bp = c // 2, c % 2
        hs = slice(hp * 128, (hp + 1) * 128)
        # per-chunk [128 rows, 128 cols] blocks: all wq, then wk, then wv
        wq = np.concatenate(
            [Wq[ci * 128:(ci + 1) * 128, hs] for ci in range(NCH)], axis=1
        )
        wk = np.concatenate(
            [Wk[ci * 128:(ci + 1) * 128, hs] for ci in range(NCH)], axis=1
        )
        wv = np.concatenate(
            [Wv[ci * 128:(ci + 1) * 128, hs] for ci in range(NCH)], axis=1
        )
        wp = np.concatenate([wq, wk, wv, eye], axis=1)
        wo2 = np.concatenate(
            [Wo[hp * 128:hp * 128 + 64, :], Wo[hp * 128 + 64:hp * 128 + 128, :]],
            axis=1,
        )
        in_maps.append({
            "xT": np.ascontiguousarray(xT[2 * bp:2 * bp + 2]),
            "wpack": np.ascontiguousarray(wp).astype(BF),
            "wo": np.ascontiguousarray(wo2).astype(BF),
            "bqk": np.ascontiguousarray(
                np.stack([bq[hs], bk[hs]], axis=1)).astype(np.float32),
        })

    res = run_bass_kernel_spmd(nc, in_maps, list(range(NCORES)))

    acc = np.zeros((B, S, D), dtype=np.float32)
    for c in range(NCORES):
        hp, bp = c // 2, c % 2
        o = np.asarray(res.results[c]["out"]).astype(np.float32)
        o = o.reshape(2, NB, S, D)
        d = np.asarray(res.results[c]["dnm"]).astype(np.float32)
        for h in range(2):
            for lb in range(NB):
                acc[2 * bp + lb] += o[h, lb] / d[h, lb][:, None]
    # biases that commute with the head-reduction, applied at gather time
    acc += bo[None, :] + (bv @ Wo)[None, :]
    return acc
